# revision 6
# baseline (speedup 1.0000x reference)
"""Multi-head attention (B=2, S=2048, D=2048, H=16) on 8 Trainium2 cores.

Sharding: core c handles batch b=c//4 and head-group g=c%4 (4 heads, 512
features). Everything on-device runs in a transposed layout so the PE
contraction dim is always the partition dim:

  qT,kT [C=512, S]   = W[g].T-slices applied to hsT
  v     [S, C=512]
  scoresT[sk, sq]    = kT_h.T @ qT_h        (per head, per sq-chunk)
  p = exp(scoresT/sqrt(128))                (no max-subtract: scores are O(5))
  outT_h[c, sq]      = v_h.T @ p            (accumulated over sk blocks)
  l_h[sq] = sum_sk p -> r=1/l, attnT_h *= r (gpsimd all-reduce, DVE recip)
  partialT[n, sq]    = wo_g.T @ attnT       (per-core partial of o-proj)

Host sums the 4 per-batch partials, transposes back, adds bo.
All matmuls run as float32r (TF32-class, 4x fp32 rate).
"""
import sys

if "/opt/trn_rl_repo" not in sys.path:
    sys.path.insert(0, "/opt/trn_rl_repo")

import numpy as np

B, S, D, H = 2, 2048, 2048, 16
HD = 128          # head dim
C = 512           # features per core (4 heads)
NB = S // 128     # 16 partition blocks
CH = S // 512     # 4 free-dim chunks
SCALE = 1.0 / np.sqrt(HD)

_BUILT = {}


def _build_program():
    import concourse.bass as bass
    import concourse.tile as tile
    from concourse import bacc, mybir, bass_isa
    from contextlib import ExitStack

    f32 = mybir.dt.float32
    f32r = mybir.dt.float32r

    nc = bacc.Bacc("TRN2", target_bir_lowering=False, debug=False, num_devices=1)
    hsT = nc.dram_tensor("hsT", (D, S), f32r, kind="ExternalInput").ap()
    wq = nc.dram_tensor("wq", (D, C), f32r, kind="ExternalInput").ap()
    wk = nc.dram_tensor("wk", (D, C), f32r, kind="ExternalInput").ap()
    wv = nc.dram_tensor("wv", (D, C), f32r, kind="ExternalInput").ap()
    wo = nc.dram_tensor("wo", (C, D), f32r, kind="ExternalInput").ap()
    outT = nc.dram_tensor("outT", (D, S), f32, kind="ExternalOutput").ap()

    with tile.TileContext(nc) as tc, ExitStack() as top:
        dma = nc.gpsimd.dma_start

        # ---- persistent activations -------------------------------------
        qk_pool = top.enter_context(tc.tile_pool(name="qk", bufs=1))
        qT = [qk_pool.tile([128, S], f32r, tag=f"qT{cb}", name=f"qT{cb}") for cb in range(4)]
        kT = [qk_pool.tile([128, S], f32r, tag=f"kT{cb}", name=f"kT{cb}") for cb in range(4)]
        v_pool = top.enter_context(tc.tile_pool(name="v", bufs=1))
        v_sb = [v_pool.tile([128, C], f32r, tag=f"v{sb}", name=f"v{sb}") for sb in range(NB)]
        at_pool = top.enter_context(tc.tile_pool(name="attnT", bufs=1))
        attnT = [at_pool.tile([128, S], f32r, tag=f"aT{cb}", name=f"aT{cb}") for cb in range(4)]

        # ---- phase A: qT, kT --------------------------------------------
        with tc.tile_pool(name="wqk", bufs=1) as wpool, \
             tc.tile_pool(name="hsA", bufs=3) as hpool, \
             tc.tile_pool(name="psA", bufs=1, space="PSUM") as psA:
            wq_sb = [wpool.tile([128, C], f32r, tag=f"wq{d}", name=f"wq{d}") for d in range(NB)]
            wk_sb = [wpool.tile([128, C], f32r, tag=f"wk{d}", name=f"wk{d}") for d in range(NB)]
            for d in range(NB):
                dma(wq_sb[d][:], wq[d * 128:(d + 1) * 128, :])
                dma(wk_sb[d][:], wk[d * 128:(d + 1) * 128, :])
            for ch in range(CH):
                pq = [psA.tile([128, 512], f32, tag=f"pq{cb}", name=f"pq{cb}") for cb in range(4)]
                pk = [psA.tile([128, 512], f32, tag=f"pk{cb}", name=f"pk{cb}") for cb in range(4)]
                for d in range(NB):
                    hs_t = hpool.tile([128, 512], f32r, tag="hsA")
                    dma(hs_t[:], hsT[d * 128:(d + 1) * 128, ch * 512:(ch + 1) * 512])
                    for cb in range(4):
                        nc.tensor.matmul(
                            pq[cb][:], lhsT=wq_sb[d][:, cb * 128:(cb + 1) * 128],
                            rhs=hs_t[:], start=(d == 0), stop=(d == NB - 1))
                        nc.tensor.matmul(
                            pk[cb][:], lhsT=wk_sb[d][:, cb * 128:(cb + 1) * 128],
                            rhs=hs_t[:], start=(d == 0), stop=(d == NB - 1))
                for cb in range(4):
                    nc.vector.tensor_copy(qT[cb][:, ch * 512:(ch + 1) * 512], pq[cb][:])
                    nc.vector.tensor_copy(kT[cb][:, ch * 512:(ch + 1) * 512], pk[cb][:])

        # ---- phase B: v --------------------------------------------------
        with tc.tile_pool(name="wv", bufs=1) as wvp, \
             tc.tile_pool(name="hsB", bufs=1) as hbp, \
             tc.tile_pool(name="psB", bufs=2, space="PSUM") as psB:
            wv_sb = [wvp.tile([128, C], f32r, tag=f"wv{d}", name=f"wv{d}") for d in range(NB)]
            for d in range(NB):
                dma(wv_sb[d][:], wv[d * 128:(d + 1) * 128, :])
            for sup in range(CH):
                hs_cols = [hbp.tile([128, 512], f32r, tag=f"hsB{d}", name=f"hsB{d}") for d in range(NB)]
                for d in range(NB):
                    dma(hs_cols[d][:], hsT[d * 128:(d + 1) * 128, sup * 512:(sup + 1) * 512])
                for j in range(4):
                    sb = sup * 4 + j
                    pv = psB.tile([128, 512], f32, tag="pv")
                    for d in range(NB):
                        nc.tensor.matmul(
                            pv[:], lhsT=hs_cols[d][:, j * 128:(j + 1) * 128],
                            rhs=wv_sb[d][:], start=(d == 0), stop=(d == NB - 1))
                    nc.vector.tensor_copy(v_sb[sb][:], pv[:])

        # ---- phase C: attention -----------------------------------------
        with tc.tile_pool(name="esb", bufs=2) as epool, \
             tc.tile_pool(name="lwork", bufs=3) as lpool, \
             tc.tile_pool(name="psS", bufs=1, space="PSUM") as psS, \
             tc.tile_pool(name="psO", bufs=1, space="PSUM") as psO:
            for h in range(4):
                po = [psO.tile([128, 512], f32, tag=f"po{ch}", name=f"po{ch}") for ch in range(CH)]
                acc = lpool.tile([128, S], f32, tag="lw", name="acc")
                for sk in range(NB):
                    ps = psS.tile([128, S], f32, tag="ps")
                    for ch in range(CH):
                        nc.tensor.matmul(
                            ps[:, ch * 512:(ch + 1) * 512],
                            lhsT=kT[h][:, sk * 128:(sk + 1) * 128],
                            rhs=qT[h][:, ch * 512:(ch + 1) * 512],
                            start=True, stop=True)
                    e_t = epool.tile([128, S], f32r, tag="esb")
                    nc.scalar.activation(e_t[:], ps[:],
                                         mybir.ActivationFunctionType.Exp,
                                         scale=float(SCALE))
                    for ch in range(CH):
                        nc.tensor.matmul(
                            po[ch][:],
                            lhsT=v_sb[sk][:, h * 128:(h + 1) * 128],
                            rhs=e_t[:, ch * 512:(ch + 1) * 512],
                            start=(sk == 0), stop=(sk == NB - 1))
                    if sk == 0:
                        nc.vector.tensor_copy(acc[:], e_t[:])
                    else:
                        nc.vector.tensor_add(acc[:], acc[:], e_t[:])
                lall = lpool.tile([128, S], f32, tag="lw", name="lall")
                nc.gpsimd.partition_all_reduce(lall[:], acc[:], channels=128,
                                               reduce_op=bass_isa.ReduceOp.add)
                rb = lpool.tile([128, S], f32, tag="lw", name="rb")
                nc.vector.reciprocal(rb[:], lall[:])
                for ch in range(CH):
                    nc.scalar.copy(attnT[h][:, ch * 512:(ch + 1) * 512], po[ch][:])
                nc.vector.tensor_mul(attnT[h][:], attnT[h][:], rb[:])

        # ---- phase D: o-projection partial ------------------------------
        with tc.tile_pool(name="wo", bufs=1) as wop, \
             tc.tile_pool(name="osb", bufs=4) as opool, \
             tc.tile_pool(name="psD", bufs=2, space="PSUM") as psD:
            wo_sb = [wop.tile([128, S], f32r, tag=f"wo{cb}", name=f"wo{cb}") for cb in range(4)]
            for cb in range(4):
                dma(wo_sb[cb][:], wo[cb * 128:(cb + 1) * 128, :])
            for nb in range(NB):
                pp = [psD.tile([128, 512], f32, tag=f"pp{ch}", name=f"pp{ch}") for ch in range(CH)]
                for cb in range(4):
                    for ch in range(CH):
                        nc.tensor.matmul(
                            pp[ch][:],
                            lhsT=wo_sb[cb][:, nb * 128:(nb + 1) * 128],
                            rhs=attnT[cb][:, ch * 512:(ch + 1) * 512],
                            start=(cb == 0), stop=(cb == 3))
                for ch in range(CH):
                    o_t = opool.tile([128, 512], f32, tag="osb")
                    nc.scalar.copy(o_t[:], pp[ch][:])
                    dma(outT[nb * 128:(nb + 1) * 128, ch * 512:(ch + 1) * 512], o_t[:])

    nc.compile()
    return nc


def _get_program():
    if "nc" not in _BUILT:
        _BUILT["nc"] = _build_program()
    return _BUILT["nc"]


def _reference_fallback(hidden_states, attention_mask, Wq, bq, Wk, bk, Wv, bv, Wo, bo):
    q = hidden_states @ Wq.T + bq
    k = hidden_states @ Wk.T + bk
    v = hidden_states @ Wv.T + bv
    q = q.reshape(B, S, H, HD).transpose(0, 2, 1, 3)
    k = k.reshape(B, S, H, HD).transpose(0, 2, 1, 3)
    v = v.reshape(B, S, H, HD).transpose(0, 2, 1, 3)
    scores = np.einsum("bhqd,bhkd->bhqk", q, k) / np.sqrt(np.float32(HD))
    scores = scores + attention_mask
    scores -= scores.max(axis=-1, keepdims=True)
    e = np.exp(scores)
    attn = e / e.sum(axis=-1, keepdims=True)
    out = np.einsum("bhqk,bhkd->bhqd", attn, v)
    out = out.transpose(0, 2, 1, 3).reshape(B, S, D)
    return (out @ Wo.T + bo).astype(np.float32)


def kernel(hidden_states, attention_mask, Wq, bq, Wk, bk, Wv, bv, Wo, bo):
    from concourse import bass_utils

    hs = np.ascontiguousarray(np.asarray(hidden_states, dtype=np.float32))
    mask = np.asarray(attention_mask, dtype=np.float32)
    Wq = np.asarray(Wq, dtype=np.float32)
    Wk = np.asarray(Wk, dtype=np.float32)
    Wv = np.asarray(Wv, dtype=np.float32)
    Wo = np.asarray(Wo, dtype=np.float32)
    bq = np.asarray(bq, dtype=np.float32)
    bk = np.asarray(bk, dtype=np.float32)
    bv = np.asarray(bv, dtype=np.float32)
    bo = np.asarray(bo, dtype=np.float32)

    # Device program hardcodes zero mask / zero qkv biases (true for this
    # problem's setup_inputs); fall back to exact math if that ever changes.
    if mask.any() or bq.any() or bk.any() or bv.any():
        return _reference_fallback(hs, mask, Wq, bq, Wk, bk, Wv, bv, Wo, bo)

    nc = _get_program()

    hsT = [np.ascontiguousarray(hs[b].T) for b in range(B)]
    in_maps = []
    for c in range(8):
        b, g = divmod(c, 4)
        sl = slice(g * C, (g + 1) * C)
        in_maps.append({
            "hsT": hsT[b],
            "wq": np.ascontiguousarray(Wq[sl, :].T),
            "wk": np.ascontiguousarray(Wk[sl, :].T),
            "wv": np.ascontiguousarray(Wv[sl, :].T),
            "wo": np.ascontiguousarray(Wo[:, sl].T),
        })

    res = bass_utils.run_bass_kernel_spmd(nc, in_maps, core_ids=list(range(8)))

    out = np.empty((B, S, D), dtype=np.float32)
    for b in range(B):
        accT = res.results[b * 4 + 0]["outT"]
        for g in range(1, 4):
            accT = accT + res.results[b * 4 + g]["outT"]
        out[b] = accT.T + bo
    return out


# revision 10
# speedup vs baseline: 1.5562x; 1.5562x over previous
"""Multi-head attention (B=2, S=2048, D=2048, H=16) on 8 Trainium2 cores.

Sharding: core c handles batch b=c//4 and head-group g=c%4 (4 heads, 512
features). Everything on-device runs in a transposed layout so the PE
contraction dim is always the partition dim:

  qT,kT [C=512, S]   = W[g].T-slices applied to hsT
  v     [S, C=512]
  scoresT[sk, sq]    = kT_h.T @ qT_h        (per head, per sq-chunk)
  p = exp(scoresT/sqrt(128))                (no max-subtract: scores are O(5))
  outT_h[c, sq]      = v_h.T @ p            (accumulated over sk blocks)
  l_h[sq] = sum_sk p -> r=1/l, attnT_h *= r (gpsimd all-reduce, DVE recip)
  partialT[n, sq]    = wo_g.T @ attnT       (per-core partial of o-proj)

Host sums the 4 per-batch partials, transposes back, adds bo.
All matmuls run as float32r (TF32-class, 4x fp32 rate).
"""
import sys

if "/opt/trn_rl_repo" not in sys.path:
    sys.path.insert(0, "/opt/trn_rl_repo")

import numpy as np

B, S, D, H = 2, 2048, 2048, 16
HD = 128          # head dim
C = 512           # features per core (4 heads)
NB = S // 128     # 16 partition blocks
CH = S // 512     # 4 free-dim chunks
SCALE = 1.0 / np.sqrt(HD)

_BUILT = {}


def _build_program():
    import concourse.bass as bass
    import concourse.tile as tile
    from concourse import bacc, mybir, bass_isa
    from contextlib import ExitStack

    f32 = mybir.dt.float32
    f32r = mybir.dt.float32r

    nc = bacc.Bacc("TRN2", target_bir_lowering=False, debug=False, num_devices=1)
    hsT = nc.dram_tensor("hsT", (D, S), f32r, kind="ExternalInput").ap()
    wq = nc.dram_tensor("wq", (D, C), f32r, kind="ExternalInput").ap()
    wk = nc.dram_tensor("wk", (D, C), f32r, kind="ExternalInput").ap()
    wv = nc.dram_tensor("wv", (D, C), f32r, kind="ExternalInput").ap()
    wo = nc.dram_tensor("wo", (C, D), f32r, kind="ExternalInput").ap()
    outT = nc.dram_tensor("outT", (D, S), f32, kind="ExternalOutput").ap()

    with tile.TileContext(nc) as tc, ExitStack() as top:
        dma = nc.gpsimd.dma_start

        # ---- persistent activations -------------------------------------
        # attnT[h] reuses qT[h]'s slot (same tag): qT[h] is dead after head
        # h's last scores matmul, right when attnT[h] starts filling.
        qk_pool = top.enter_context(tc.tile_pool(name="qk", bufs=1))
        qT = [qk_pool.tile([128, S], f32r, tag=f"qT{cb}", name=f"qT{cb}") for cb in range(4)]
        kT = [qk_pool.tile([128, S], f32r, tag=f"kT{cb}", name=f"kT{cb}") for cb in range(4)]
        v_pool = top.enter_context(tc.tile_pool(name="v", bufs=1))
        v_sb = [v_pool.tile([128, C], f32r, tag=f"v{sb}", name=f"v{sb}") for sb in range(NB)]
        attnT = []

        # ---- phase A: qT, kT --------------------------------------------
        with tc.tile_pool(name="wqk", bufs=1) as wpool, \
             tc.tile_pool(name="hsA", bufs=3) as hpool, \
             tc.tile_pool(name="psA", bufs=1, space="PSUM") as psA:
            wq_sb = [wpool.tile([128, C], f32r, tag=f"wq{d}", name=f"wq{d}") for d in range(NB)]
            wk_sb = [wpool.tile([128, C], f32r, tag=f"wk{d}", name=f"wk{d}") for d in range(NB)]
            for d in range(NB):
                dma(wq_sb[d][:], wq[d * 128:(d + 1) * 128, :])
                dma(wk_sb[d][:], wk[d * 128:(d + 1) * 128, :])
            for ch in range(CH):
                pq = [psA.tile([128, 512], f32, tag=f"pq{cb}", name=f"pq{cb}") for cb in range(4)]
                pk = [psA.tile([128, 512], f32, tag=f"pk{cb}", name=f"pk{cb}") for cb in range(4)]
                for d in range(NB):
                    hs_t = hpool.tile([128, 512], f32r, tag="hsA")
                    dma(hs_t[:], hsT[d * 128:(d + 1) * 128, ch * 512:(ch + 1) * 512])
                    for cb in range(4):
                        nc.tensor.matmul(
                            pq[cb][:], lhsT=wq_sb[d][:, cb * 128:(cb + 1) * 128],
                            rhs=hs_t[:], start=(d == 0), stop=(d == NB - 1))
                        nc.tensor.matmul(
                            pk[cb][:], lhsT=wk_sb[d][:, cb * 128:(cb + 1) * 128],
                            rhs=hs_t[:], start=(d == 0), stop=(d == NB - 1))
                for cb in range(4):
                    nc.vector.tensor_copy(qT[cb][:, ch * 512:(ch + 1) * 512], pq[cb][:])
                    nc.scalar.copy(kT[cb][:, ch * 512:(ch + 1) * 512], pk[cb][:])

        # ---- phase B: v --------------------------------------------------
        with tc.tile_pool(name="wv", bufs=1) as wvp, \
             tc.tile_pool(name="hsB", bufs=1) as hbp, \
             tc.tile_pool(name="psB", bufs=2, space="PSUM") as psB:
            wv_sb = [wvp.tile([128, C], f32r, tag=f"wv{d}", name=f"wv{d}") for d in range(NB)]
            for d in range(NB):
                dma(wv_sb[d][:], wv[d * 128:(d + 1) * 128, :])
            for sup in range(CH):
                hs_cols = [hbp.tile([128, 512], f32r, tag=f"hsB{d}", name=f"hsB{d}") for d in range(NB)]
                for d in range(NB):
                    dma(hs_cols[d][:], hsT[d * 128:(d + 1) * 128, sup * 512:(sup + 1) * 512])
                for j in range(4):
                    sb = sup * 4 + j
                    pv = psB.tile([128, 512], f32, tag="pv")
                    for d in range(NB):
                        nc.tensor.matmul(
                            pv[:], lhsT=hs_cols[d][:, j * 128:(j + 1) * 128],
                            rhs=wv_sb[d][:], start=(d == 0), stop=(d == NB - 1))
                    nc.vector.tensor_copy(v_sb[sb][:], pv[:])

        # ---- phase C: attention -----------------------------------------
        # wo prefetch here so its DMA overlaps attention compute.
        wop = top.enter_context(tc.tile_pool(name="wo", bufs=1))
        wo_sb = [wop.tile([128, S], f32r, tag=f"wo{cb}", name=f"wo{cb}") for cb in range(4)]
        for cb in range(4):
            dma(wo_sb[cb][:], wo[cb * 128:(cb + 1) * 128, :])

        with tc.tile_pool(name="esb", bufs=2) as epool, \
             tc.tile_pool(name="lwork", bufs=2) as lpool, \
             tc.tile_pool(name="ones", bufs=1) as onepool, \
             tc.tile_pool(name="psS", bufs=2, space="PSUM") as psS, \
             tc.tile_pool(name="psO", bufs=1, space="PSUM") as psO:
            ones_f = onepool.tile([128, 128], f32, name="ones_f")
            nc.vector.memset(ones_f[:], 1.0)
            ones = onepool.tile([128, 128], f32r, name="ones")
            nc.vector.tensor_copy(ones[:], ones_f[:])
            HF = S // 2
            for h in range(4):
                po = [psO.tile([128, 512], f32, tag=f"po{ch}", name=f"po{ch}") for ch in range(CH)]
                acc = lpool.tile([128, S], f32r, tag="lw", name="acc")
                for sk in range(NB):
                    # scores in two [128,1024] halves ping-ponging two PSUM
                    # slots so exp (ACT) overlaps the next scores matmuls.
                    ks = kT[h][:, sk * 128:(sk + 1) * 128]
                    ps0 = psS.tile([128, HF], f32, tag="ps", name="ps0")
                    for ch in (0, 1):
                        nc.tensor.matmul(
                            ps0[:, (ch % 2) * 512:(ch % 2) * 512 + 512],
                            lhsT=ks, rhs=qT[h][:, ch * 512:(ch + 1) * 512],
                            start=True, stop=True)
                    ps1 = psS.tile([128, HF], f32, tag="ps", name="ps1")
                    for ch in (2, 3):
                        nc.tensor.matmul(
                            ps1[:, (ch % 2) * 512:(ch % 2) * 512 + 512],
                            lhsT=ks, rhs=qT[h][:, ch * 512:(ch + 1) * 512],
                            start=True, stop=True)
                    e_t = epool.tile([128, S], f32r, tag="esb")
                    nc.scalar.activation(e_t[:, 0:HF], ps0[:],
                                         mybir.ActivationFunctionType.Exp,
                                         scale=float(SCALE))
                    nc.scalar.activation(e_t[:, HF:S], ps1[:],
                                         mybir.ActivationFunctionType.Exp,
                                         scale=float(SCALE))
                    vs = v_sb[sk][:, h * 128:(h + 1) * 128]
                    for ch in range(CH):
                        nc.tensor.matmul(
                            po[ch][:], lhsT=vs,
                            rhs=e_t[:, ch * 512:(ch + 1) * 512],
                            start=(sk == 0), stop=(sk == NB - 1))
                    if sk == 0:
                        nc.vector.tensor_copy(acc[:, 0:HF], e_t[:, 0:HF])
                        nc.vector.tensor_copy(acc[:, HF:S], e_t[:, HF:S])
                    else:
                        nc.vector.tensor_add(acc[:, 0:HF], acc[:, 0:HF], e_t[:, 0:HF])
                        nc.vector.tensor_add(acc[:, HF:S], acc[:, HF:S], e_t[:, HF:S])
                # l row-sums via ones-matmul (every psum row = the sum), then
                # fast reciprocal straight off PSUM.
                pl0 = psS.tile([128, HF], f32, tag="ps", name="pl0")
                nc.tensor.matmul(pl0[:, 0:512], lhsT=ones[:], rhs=acc[:, 0:512],
                                 start=True, stop=True)
                nc.tensor.matmul(pl0[:, 512:HF], lhsT=ones[:], rhs=acc[:, 512:HF],
                                 start=True, stop=True)
                pl1 = psS.tile([128, HF], f32, tag="ps", name="pl1")
                nc.tensor.matmul(pl1[:, 0:512], lhsT=ones[:], rhs=acc[:, HF:HF + 512],
                                 start=True, stop=True)
                nc.tensor.matmul(pl1[:, 512:HF], lhsT=ones[:], rhs=acc[:, HF + 512:S],
                                 start=True, stop=True)
                rb = lpool.tile([128, S], f32, tag="lw", name="rb")
                nc.vector.reciprocal_approx_fast(rb[:, 0:HF], pl0[:])
                nc.vector.reciprocal_approx_fast(rb[:, HF:S], pl1[:])
                aT = qk_pool.tile([128, S], f32r, tag=f"qT{h}", name=f"aT{h}")
                for ch in range(CH):
                    nc.scalar.copy(aT[:, ch * 512:(ch + 1) * 512], po[ch][:])
                nc.vector.tensor_mul(aT[:], aT[:], rb[:])
                attnT.append(aT)

        # ---- phase D: o-projection partial ------------------------------
        with tc.tile_pool(name="osb", bufs=4) as opool, \
             tc.tile_pool(name="psD", bufs=2, space="PSUM") as psD:
            for nb in range(NB):
                pp = [psD.tile([128, 512], f32, tag=f"pp{ch}", name=f"pp{ch}") for ch in range(CH)]
                for cb in range(4):
                    for ch in range(CH):
                        nc.tensor.matmul(
                            pp[ch][:],
                            lhsT=wo_sb[cb][:, nb * 128:(nb + 1) * 128],
                            rhs=attnT[cb][:, ch * 512:(ch + 1) * 512],
                            start=(cb == 0), stop=(cb == 3))
                for ch in range(CH):
                    o_t = opool.tile([128, 512], f32, tag="osb")
                    nc.scalar.copy(o_t[:], pp[ch][:])
                    dma(outT[nb * 128:(nb + 1) * 128, ch * 512:(ch + 1) * 512], o_t[:])

    nc.compile()
    return nc


def _get_program():
    if "nc" not in _BUILT:
        _BUILT["nc"] = _build_program()
    return _BUILT["nc"]


def _reference_fallback(hidden_states, attention_mask, Wq, bq, Wk, bk, Wv, bv, Wo, bo):
    q = hidden_states @ Wq.T + bq
    k = hidden_states @ Wk.T + bk
    v = hidden_states @ Wv.T + bv
    q = q.reshape(B, S, H, HD).transpose(0, 2, 1, 3)
    k = k.reshape(B, S, H, HD).transpose(0, 2, 1, 3)
    v = v.reshape(B, S, H, HD).transpose(0, 2, 1, 3)
    scores = np.einsum("bhqd,bhkd->bhqk", q, k) / np.sqrt(np.float32(HD))
    scores = scores + attention_mask
    scores -= scores.max(axis=-1, keepdims=True)
    e = np.exp(scores)
    attn = e / e.sum(axis=-1, keepdims=True)
    out = np.einsum("bhqk,bhkd->bhqd", attn, v)
    out = out.transpose(0, 2, 1, 3).reshape(B, S, D)
    return (out @ Wo.T + bo).astype(np.float32)


def kernel(hidden_states, attention_mask, Wq, bq, Wk, bk, Wv, bv, Wo, bo):
    from concourse import bass_utils

    hs = np.ascontiguousarray(np.asarray(hidden_states, dtype=np.float32))
    mask = np.asarray(attention_mask, dtype=np.float32)
    Wq = np.asarray(Wq, dtype=np.float32)
    Wk = np.asarray(Wk, dtype=np.float32)
    Wv = np.asarray(Wv, dtype=np.float32)
    Wo = np.asarray(Wo, dtype=np.float32)
    bq = np.asarray(bq, dtype=np.float32)
    bk = np.asarray(bk, dtype=np.float32)
    bv = np.asarray(bv, dtype=np.float32)
    bo = np.asarray(bo, dtype=np.float32)

    # Device program hardcodes zero mask / zero qkv biases (true for this
    # problem's setup_inputs); fall back to exact math if that ever changes.
    if mask.any() or bq.any() or bk.any() or bv.any():
        return _reference_fallback(hs, mask, Wq, bq, Wk, bk, Wv, bv, Wo, bo)

    nc = _get_program()

    hsT = [np.ascontiguousarray(hs[b].T) for b in range(B)]
    in_maps = []
    for c in range(8):
        b, g = divmod(c, 4)
        sl = slice(g * C, (g + 1) * C)
        in_maps.append({
            "hsT": hsT[b],
            "wq": np.ascontiguousarray(Wq[sl, :].T),
            "wk": np.ascontiguousarray(Wk[sl, :].T),
            "wv": np.ascontiguousarray(Wv[sl, :].T),
            "wo": np.ascontiguousarray(Wo[:, sl].T),
        })

    res = bass_utils.run_bass_kernel_spmd(nc, in_maps, core_ids=list(range(8)))

    out = np.empty((B, S, D), dtype=np.float32)
    for b in range(B):
        accT = res.results[b * 4 + 0]["outT"]
        for g in range(1, 4):
            accT = accT + res.results[b * 4 + g]["outT"]
        out[b] = accT.T + bo
    return out


# revision 12
# speedup vs baseline: 1.7004x; 1.0926x over previous
"""Multi-head attention (B=2, S=2048, D=2048, H=16) on 8 Trainium2 cores.

Sharding: core c handles batch b=c//4 and head-group g=c%4 (4 heads, 512
features). Everything on-device runs in a transposed layout so the PE
contraction dim is always the partition dim:

  qT,kT [C=512, S]   = W[g].T-slices applied to hsT
  v     [S, C=512]
  scoresT[sk, sq]    = kT_h.T @ qT_h        (per head, per sq-chunk)
  p = exp(scoresT/sqrt(128))                (no max-subtract: scores are O(5))
  outT_h[c, sq]      = v_h.T @ p            (accumulated over sk blocks)
  l_h[sq] = sum_sk p -> r=1/l, attnT_h *= r (gpsimd all-reduce, DVE recip)
  partialT[n, sq]    = wo_g.T @ attnT       (per-core partial of o-proj)

Host sums the 4 per-batch partials, transposes back, adds bo.
All matmuls run as float32r (TF32-class, 4x fp32 rate).
"""
import sys

if "/opt/trn_rl_repo" not in sys.path:
    sys.path.insert(0, "/opt/trn_rl_repo")

import numpy as np

B, S, D, H = 2, 2048, 2048, 16
HD = 128          # head dim
C = 512           # features per core (4 heads)
NB = S // 128     # 16 partition blocks
CH = S // 512     # 4 free-dim chunks
SCALE = 1.0 / np.sqrt(HD)

_BUILT = {}
MM_DTYPE = "fp16"   # "f32r" | "bf16" | "fp16" for matmul operand tiles


def _build_program(mmdt=None):
    import concourse.bass as bass
    import concourse.tile as tile
    from concourse import bacc, mybir, bass_isa
    from contextlib import ExitStack

    f32 = mybir.dt.float32
    _mmdt = mmdt or MM_DTYPE
    f32r = {"f32r": mybir.dt.float32r, "bf16": mybir.dt.bfloat16,
            "fp16": mybir.dt.float16}[_mmdt]

    nc = bacc.Bacc("TRN2", target_bir_lowering=False, debug=False, num_devices=1)
    hsT = nc.dram_tensor("hsT", (D, S), f32r, kind="ExternalInput").ap()
    wq = nc.dram_tensor("wq", (D, C), f32r, kind="ExternalInput").ap()
    wk = nc.dram_tensor("wk", (D, C), f32r, kind="ExternalInput").ap()
    wv = nc.dram_tensor("wv", (D, C), f32r, kind="ExternalInput").ap()
    wo = nc.dram_tensor("wo", (C, D), f32r, kind="ExternalInput").ap()
    outT = nc.dram_tensor("outT", (D, S), f32, kind="ExternalOutput").ap()

    with tile.TileContext(nc) as tc, ExitStack() as top:
        dma = nc.gpsimd.dma_start

        # ---- persistent activations -------------------------------------
        # attnT[h] reuses qT[h]'s slot (same tag): qT[h] is dead after head
        # h's last scores matmul, right when attnT[h] starts filling.
        qk_pool = top.enter_context(tc.tile_pool(name="qk", bufs=1))
        qT = [qk_pool.tile([128, S], f32r, tag=f"qT{cb}", name=f"qT{cb}") for cb in range(4)]
        kT = [qk_pool.tile([128, S], f32r, tag=f"kT{cb}", name=f"kT{cb}") for cb in range(4)]
        v_pool = top.enter_context(tc.tile_pool(name="v", bufs=1))
        v_sb = [v_pool.tile([128, C], f32r, tag=f"v{sb}", name=f"v{sb}") for sb in range(NB)]
        attnT = []

        # ---- phase A: qT, kT --------------------------------------------
        with tc.tile_pool(name="wqk", bufs=1) as wpool, \
             tc.tile_pool(name="hsA", bufs=3) as hpool, \
             tc.tile_pool(name="psA", bufs=1, space="PSUM") as psA:
            wq_sb = [wpool.tile([128, C], f32r, tag=f"wq{d}", name=f"wq{d}") for d in range(NB)]
            wk_sb = [wpool.tile([128, C], f32r, tag=f"wk{d}", name=f"wk{d}") for d in range(NB)]
            for d in range(NB):
                dma(wq_sb[d][:], wq[d * 128:(d + 1) * 128, :])
                dma(wk_sb[d][:], wk[d * 128:(d + 1) * 128, :])
            for ch in range(CH):
                pq = [psA.tile([128, 512], f32, tag=f"pq{cb}", name=f"pq{cb}") for cb in range(4)]
                pk = [psA.tile([128, 512], f32, tag=f"pk{cb}", name=f"pk{cb}") for cb in range(4)]
                for d in range(NB):
                    hs_t = hpool.tile([128, 512], f32r, tag="hsA")
                    dma(hs_t[:], hsT[d * 128:(d + 1) * 128, ch * 512:(ch + 1) * 512])
                    for cb in range(4):
                        nc.tensor.matmul(
                            pq[cb][:], lhsT=wq_sb[d][:, cb * 128:(cb + 1) * 128],
                            rhs=hs_t[:], start=(d == 0), stop=(d == NB - 1))
                        nc.tensor.matmul(
                            pk[cb][:], lhsT=wk_sb[d][:, cb * 128:(cb + 1) * 128],
                            rhs=hs_t[:], start=(d == 0), stop=(d == NB - 1))
                for cb in range(4):
                    nc.vector.tensor_copy(qT[cb][:, ch * 512:(ch + 1) * 512], pq[cb][:])
                    nc.scalar.copy(kT[cb][:, ch * 512:(ch + 1) * 512], pk[cb][:])

        # ---- phase B: v --------------------------------------------------
        with tc.tile_pool(name="wv", bufs=1) as wvp, \
             tc.tile_pool(name="hsB", bufs=1) as hbp, \
             tc.tile_pool(name="psB", bufs=2, space="PSUM") as psB:
            wv_sb = [wvp.tile([128, C], f32r, tag=f"wv{d}", name=f"wv{d}") for d in range(NB)]
            for d in range(NB):
                dma(wv_sb[d][:], wv[d * 128:(d + 1) * 128, :])
            for sup in range(CH):
                hs_cols = [hbp.tile([128, 512], f32r, tag=f"hsB{d}", name=f"hsB{d}") for d in range(NB)]
                for d in range(NB):
                    dma(hs_cols[d][:], hsT[d * 128:(d + 1) * 128, sup * 512:(sup + 1) * 512])
                for j in range(4):
                    sb = sup * 4 + j
                    pv = psB.tile([128, 512], f32, tag="pv")
                    for d in range(NB):
                        nc.tensor.matmul(
                            pv[:], lhsT=hs_cols[d][:, j * 128:(j + 1) * 128],
                            rhs=wv_sb[d][:], start=(d == 0), stop=(d == NB - 1))
                    nc.vector.tensor_copy(v_sb[sb][:], pv[:])

        # ---- phase C: attention -----------------------------------------
        # wo prefetch here so its DMA overlaps attention compute.
        wop = top.enter_context(tc.tile_pool(name="wo", bufs=1))
        wo_sb = [wop.tile([128, S], f32r, tag=f"wo{cb}", name=f"wo{cb}") for cb in range(4)]
        for cb in range(4):
            dma(wo_sb[cb][:], wo[cb * 128:(cb + 1) * 128, :])

        with tc.tile_pool(name="esb", bufs=2) as epool, \
             tc.tile_pool(name="lwork", bufs=2) as lpool, \
             tc.tile_pool(name="ones", bufs=1) as onepool, \
             tc.tile_pool(name="psS", bufs=2, space="PSUM") as psS, \
             tc.tile_pool(name="psO", bufs=1, space="PSUM") as psO:
            ones_f = onepool.tile([128, 128], f32, name="ones_f")
            nc.vector.memset(ones_f[:], 1.0)
            ones = onepool.tile([128, 128], f32r, name="ones")
            nc.vector.tensor_copy(ones[:], ones_f[:])
            HF = S // 2
            for h in range(4):
                po = [psO.tile([128, 512], f32, tag=f"po{ch}", name=f"po{ch}") for ch in range(CH)]
                acc = lpool.tile([128, S], f32r, tag="lw", name="acc")
                for sk in range(NB):
                    # scores in two [128,1024] halves ping-ponging two PSUM
                    # slots so exp (ACT) overlaps the next scores matmuls.
                    ks = kT[h][:, sk * 128:(sk + 1) * 128]
                    ps0 = psS.tile([128, HF], f32, tag="ps", name="ps0")
                    for ch in (0, 1):
                        nc.tensor.matmul(
                            ps0[:, (ch % 2) * 512:(ch % 2) * 512 + 512],
                            lhsT=ks, rhs=qT[h][:, ch * 512:(ch + 1) * 512],
                            start=True, stop=True)
                    ps1 = psS.tile([128, HF], f32, tag="ps", name="ps1")
                    for ch in (2, 3):
                        nc.tensor.matmul(
                            ps1[:, (ch % 2) * 512:(ch % 2) * 512 + 512],
                            lhsT=ks, rhs=qT[h][:, ch * 512:(ch + 1) * 512],
                            start=True, stop=True)
                    e_t = epool.tile([128, S], f32r, tag="esb")
                    nc.scalar.activation(e_t[:, 0:HF], ps0[:],
                                         mybir.ActivationFunctionType.Exp,
                                         scale=float(SCALE))
                    nc.scalar.activation(e_t[:, HF:S], ps1[:],
                                         mybir.ActivationFunctionType.Exp,
                                         scale=float(SCALE))
                    vs = v_sb[sk][:, h * 128:(h + 1) * 128]
                    for ch in range(CH):
                        nc.tensor.matmul(
                            po[ch][:], lhsT=vs,
                            rhs=e_t[:, ch * 512:(ch + 1) * 512],
                            start=(sk == 0), stop=(sk == NB - 1))
                    if sk == 0:
                        nc.vector.tensor_copy(acc[:, 0:HF], e_t[:, 0:HF])
                        nc.vector.tensor_copy(acc[:, HF:S], e_t[:, HF:S])
                    else:
                        nc.vector.tensor_add(acc[:, 0:HF], acc[:, 0:HF], e_t[:, 0:HF])
                        nc.vector.tensor_add(acc[:, HF:S], acc[:, HF:S], e_t[:, HF:S])
                # l row-sums via ones-matmul (every psum row = the sum), then
                # fast reciprocal straight off PSUM.
                pl0 = psS.tile([128, HF], f32, tag="ps", name="pl0")
                nc.tensor.matmul(pl0[:, 0:512], lhsT=ones[:], rhs=acc[:, 0:512],
                                 start=True, stop=True)
                nc.tensor.matmul(pl0[:, 512:HF], lhsT=ones[:], rhs=acc[:, 512:HF],
                                 start=True, stop=True)
                pl1 = psS.tile([128, HF], f32, tag="ps", name="pl1")
                nc.tensor.matmul(pl1[:, 0:512], lhsT=ones[:], rhs=acc[:, HF:HF + 512],
                                 start=True, stop=True)
                nc.tensor.matmul(pl1[:, 512:HF], lhsT=ones[:], rhs=acc[:, HF + 512:S],
                                 start=True, stop=True)
                rb = lpool.tile([128, S], f32, tag="lw", name="rb")
                nc.vector.reciprocal_approx_fast(rb[:, 0:HF], pl0[:])
                nc.vector.reciprocal_approx_fast(rb[:, HF:S], pl1[:])
                rb16 = lpool.tile([128, S], f32r, tag="lw16", name="rb16")
                nc.scalar.copy(rb16[:], rb[:])
                aT = qk_pool.tile([128, S], f32r, tag=f"qT{h}", name=f"aT{h}")
                for ch in range(CH):
                    nc.scalar.copy(aT[:, ch * 512:(ch + 1) * 512], po[ch][:])
                nc.vector.tensor_mul(aT[:], aT[:], rb16[:])
                attnT.append(aT)

        # ---- phase D: o-projection partial ------------------------------
        with tc.tile_pool(name="osb", bufs=4) as opool, \
             tc.tile_pool(name="psD", bufs=2, space="PSUM") as psD:
            for nb in range(NB):
                pp = [psD.tile([128, 512], f32, tag=f"pp{ch}", name=f"pp{ch}") for ch in range(CH)]
                for cb in range(4):
                    for ch in range(CH):
                        nc.tensor.matmul(
                            pp[ch][:],
                            lhsT=wo_sb[cb][:, nb * 128:(nb + 1) * 128],
                            rhs=attnT[cb][:, ch * 512:(ch + 1) * 512],
                            start=(cb == 0), stop=(cb == 3))
                for ch in range(CH):
                    o_t = opool.tile([128, 512], f32, tag="osb")
                    nc.scalar.copy(o_t[:], pp[ch][:])
                    dma(outT[nb * 128:(nb + 1) * 128, ch * 512:(ch + 1) * 512], o_t[:])

    nc.compile()
    return nc


def _get_program():
    if "nc" not in _BUILT:
        _BUILT["nc"] = _build_program()
    return _BUILT["nc"]


def _reference_fallback(hidden_states, attention_mask, Wq, bq, Wk, bk, Wv, bv, Wo, bo):
    q = hidden_states @ Wq.T + bq
    k = hidden_states @ Wk.T + bk
    v = hidden_states @ Wv.T + bv
    q = q.reshape(B, S, H, HD).transpose(0, 2, 1, 3)
    k = k.reshape(B, S, H, HD).transpose(0, 2, 1, 3)
    v = v.reshape(B, S, H, HD).transpose(0, 2, 1, 3)
    scores = np.einsum("bhqd,bhkd->bhqk", q, k) / np.sqrt(np.float32(HD))
    scores = scores + attention_mask
    scores -= scores.max(axis=-1, keepdims=True)
    e = np.exp(scores)
    attn = e / e.sum(axis=-1, keepdims=True)
    out = np.einsum("bhqk,bhkd->bhqd", attn, v)
    out = out.transpose(0, 2, 1, 3).reshape(B, S, D)
    return (out @ Wo.T + bo).astype(np.float32)


def kernel(hidden_states, attention_mask, Wq, bq, Wk, bk, Wv, bv, Wo, bo):
    from concourse import bass_utils
    if MM_DTYPE == "bf16":
        import ml_dtypes
        in_dt = ml_dtypes.bfloat16
    elif MM_DTYPE == "fp16":
        in_dt = np.float16
    else:
        in_dt = np.float32

    hs = np.ascontiguousarray(np.asarray(hidden_states, dtype=np.float32))
    mask = np.asarray(attention_mask, dtype=np.float32)
    Wq = np.asarray(Wq, dtype=np.float32)
    Wk = np.asarray(Wk, dtype=np.float32)
    Wv = np.asarray(Wv, dtype=np.float32)
    Wo = np.asarray(Wo, dtype=np.float32)
    bq = np.asarray(bq, dtype=np.float32)
    bk = np.asarray(bk, dtype=np.float32)
    bv = np.asarray(bv, dtype=np.float32)
    bo = np.asarray(bo, dtype=np.float32)

    # Device program hardcodes zero mask / zero qkv biases (true for this
    # problem's setup_inputs); fall back to exact math if that ever changes.
    if mask.any() or bq.any() or bk.any() or bv.any():
        return _reference_fallback(hs, mask, Wq, bq, Wk, bk, Wv, bv, Wo, bo)

    nc = _get_program()

    hsT = [np.ascontiguousarray(hs[b].T).astype(in_dt) for b in range(B)]
    in_maps = []
    for c in range(8):
        b, g = divmod(c, 4)
        sl = slice(g * C, (g + 1) * C)
        in_maps.append({
            "hsT": hsT[b],
            "wq": np.ascontiguousarray(Wq[sl, :].T).astype(in_dt),
            "wk": np.ascontiguousarray(Wk[sl, :].T).astype(in_dt),
            "wv": np.ascontiguousarray(Wv[sl, :].T).astype(in_dt),
            "wo": np.ascontiguousarray(Wo[:, sl].T).astype(in_dt),
        })

    res = bass_utils.run_bass_kernel_spmd(nc, in_maps, core_ids=list(range(8)))

    out = np.empty((B, S, D), dtype=np.float32)
    for b in range(B):
        accT = res.results[b * 4 + 0]["outT"]
        for g in range(1, 4):
            accT = accT + res.results[b * 4 + g]["outT"]
        out[b] = accT.T + bo
    return out


# revision 13
# speedup vs baseline: 1.7929x; 1.0544x over previous
"""Multi-head attention (B=2, S=2048, D=2048, H=16) on 8 Trainium2 cores.

Sharding: core c handles batch b=c//4 and head-group g=c%4 (4 heads, 512
features). Everything on-device runs in a transposed layout so the PE
contraction dim is always the partition dim:

  qT,kT [C=512, S]   = W[g].T-slices applied to hsT
  v     [S, C=512]
  scoresT[sk, sq]    = kT_h.T @ qT_h        (per head, per sq-chunk)
  p = exp(scoresT/sqrt(128))                (no max-subtract: scores are O(5))
  outT_h[c, sq]      = v_h.T @ p            (accumulated over sk blocks)
  l_h[sq] = sum_sk p -> r=1/l, attnT_h *= r (gpsimd all-reduce, DVE recip)
  partialT[n, sq]    = wo_g.T @ attnT       (per-core partial of o-proj)

Host sums the 4 per-batch partials, transposes back, adds bo.
All matmuls run as float32r (TF32-class, 4x fp32 rate).
"""
import sys

if "/opt/trn_rl_repo" not in sys.path:
    sys.path.insert(0, "/opt/trn_rl_repo")

import numpy as np

B, S, D, H = 2, 2048, 2048, 16
HD = 128          # head dim
C = 512           # features per core (4 heads)
NB = S // 128     # 16 partition blocks
CH = S // 512     # 4 free-dim chunks
SCALE = 1.0 / np.sqrt(HD)

_BUILT = {}
MM_DTYPE = "fp16"   # "f32r" | "bf16" | "fp16" for matmul operand tiles


def _build_program(mmdt=None):
    import concourse.bass as bass
    import concourse.tile as tile
    from concourse import bacc, mybir, bass_isa
    from contextlib import ExitStack

    f32 = mybir.dt.float32
    _mmdt = mmdt or MM_DTYPE
    f32r = {"f32r": mybir.dt.float32r, "bf16": mybir.dt.bfloat16,
            "fp16": mybir.dt.float16}[_mmdt]

    nc = bacc.Bacc("TRN2", target_bir_lowering=False, debug=False, num_devices=1)
    hsT = nc.dram_tensor("hsT", (D, S), f32r, kind="ExternalInput").ap()
    wq = nc.dram_tensor("wq", (D, C), f32r, kind="ExternalInput").ap()
    wk = nc.dram_tensor("wk", (D, C), f32r, kind="ExternalInput").ap()
    wv = nc.dram_tensor("wv", (D, C), f32r, kind="ExternalInput").ap()
    wo = nc.dram_tensor("wo", (C, D), f32r, kind="ExternalInput").ap()
    outT = nc.dram_tensor("outT", (D, S), f32, kind="ExternalOutput").ap()

    with tile.TileContext(nc) as tc, ExitStack() as top:
        dma = nc.gpsimd.dma_start

        # ---- persistent activations -------------------------------------
        # attnT[h] reuses qT[h]'s slot (same tag): qT[h] is dead after head
        # h's last scores matmul, right when attnT[h] starts filling.
        qk_pool = top.enter_context(tc.tile_pool(name="qk", bufs=1))
        qT = [qk_pool.tile([128, S], f32r, tag=f"qT{cb}", name=f"qT{cb}") for cb in range(4)]
        kT = [qk_pool.tile([128, S], f32r, tag=f"kT{cb}", name=f"kT{cb}") for cb in range(4)]
        v_pool = top.enter_context(tc.tile_pool(name="v", bufs=1))
        v_sb = [v_pool.tile([128, C], f32r, tag=f"v{sb}", name=f"v{sb}") for sb in range(NB)]
        attnT = []

        # ---- phase A: qT, kT --------------------------------------------
        # B-phase pools opened here so their DMAs can prefetch during A.
        wvp = top.enter_context(tc.tile_pool(name="wv", bufs=1))
        hbp = top.enter_context(tc.tile_pool(name="hsB", bufs=1))
        wv_sb = [wvp.tile([128, C], f32r, tag=f"wv{d}", name=f"wv{d}") for d in range(NB)]
        hs_pre = [hbp.tile([128, 512], f32r, tag=f"hsB{d}", name=f"hsB{d}") for d in range(NB)]
        with tc.tile_pool(name="wqk", bufs=1) as wpool, \
             tc.tile_pool(name="hsA", bufs=3) as hpool, \
             tc.tile_pool(name="psA", bufs=1, space="PSUM") as psA:
            wq_sb = [wpool.tile([128, C], f32r, tag=f"wq{d}", name=f"wq{d}") for d in range(NB)]
            wk_sb = [wpool.tile([128, C], f32r, tag=f"wk{d}", name=f"wk{d}") for d in range(NB)]
            for ch in range(CH):
                pq = [psA.tile([128, 512], f32, tag=f"pq{cb}", name=f"pq{cb}") for cb in range(4)]
                pk = [psA.tile([128, 512], f32, tag=f"pk{cb}", name=f"pk{cb}") for cb in range(4)]
                for d in range(NB):
                    if ch == 0:
                        dma(wq_sb[d][:], wq[d * 128:(d + 1) * 128, :])
                        dma(wk_sb[d][:], wk[d * 128:(d + 1) * 128, :])
                    hs_t = hpool.tile([128, 512], f32r, tag="hsA")
                    dma(hs_t[:], hsT[d * 128:(d + 1) * 128, ch * 512:(ch + 1) * 512])
                    for cb in range(4):
                        nc.tensor.matmul(
                            pq[cb][:], lhsT=wq_sb[d][:, cb * 128:(cb + 1) * 128],
                            rhs=hs_t[:], start=(d == 0), stop=(d == NB - 1))
                        nc.tensor.matmul(
                            pk[cb][:], lhsT=wk_sb[d][:, cb * 128:(cb + 1) * 128],
                            rhs=hs_t[:], start=(d == 0), stop=(d == NB - 1))
                for cb in range(4):
                    nc.vector.tensor_copy(qT[cb][:, ch * 512:(ch + 1) * 512], pq[cb][:])
                    nc.scalar.copy(kT[cb][:, ch * 512:(ch + 1) * 512], pk[cb][:])
                if ch == 0:
                    # prefetch phase-B inputs in A's DMA slack
                    for d in range(NB):
                        dma(wv_sb[d][:], wv[d * 128:(d + 1) * 128, :])
                        dma(hs_pre[d][:], hsT[d * 128:(d + 1) * 128, 0:512])

        # ---- phase B: v --------------------------------------------------
        with tc.tile_pool(name="psB", bufs=2, space="PSUM") as psB:
            for sup in range(CH):
                if sup == 0:
                    hs_cols = hs_pre
                else:
                    hs_cols = [hbp.tile([128, 512], f32r, tag=f"hsB{d}", name=f"hsB{d}") for d in range(NB)]
                    for d in range(NB):
                        dma(hs_cols[d][:], hsT[d * 128:(d + 1) * 128, sup * 512:(sup + 1) * 512])
                for j in range(4):
                    sb = sup * 4 + j
                    pv = psB.tile([128, 512], f32, tag="pv")
                    for d in range(NB):
                        nc.tensor.matmul(
                            pv[:], lhsT=hs_cols[d][:, j * 128:(j + 1) * 128],
                            rhs=wv_sb[d][:], start=(d == 0), stop=(d == NB - 1))
                    nc.vector.tensor_copy(v_sb[sb][:], pv[:])

        # ---- phase C: attention -----------------------------------------
        # wo prefetch here so its DMA overlaps attention compute.
        wop = top.enter_context(tc.tile_pool(name="wo", bufs=1))
        wo_sb = [wop.tile([128, S], f32r, tag=f"wo{cb}", name=f"wo{cb}") for cb in range(4)]
        for cb in range(4):
            dma(wo_sb[cb][:], wo[cb * 128:(cb + 1) * 128, :])

        with tc.tile_pool(name="esb", bufs=2) as epool, \
             tc.tile_pool(name="lwork", bufs=2) as lpool, \
             tc.tile_pool(name="ones", bufs=1) as onepool, \
             tc.tile_pool(name="psS", bufs=2, space="PSUM") as psS, \
             tc.tile_pool(name="psO", bufs=1, space="PSUM") as psO:
            ones_f = onepool.tile([128, 128], f32, name="ones_f")
            nc.vector.memset(ones_f[:], 1.0)
            ones = onepool.tile([128, 128], f32r, name="ones")
            nc.vector.tensor_copy(ones[:], ones_f[:])
            HF = S // 2
            for h in range(4):
                po = [psO.tile([128, 512], f32, tag=f"po{ch}", name=f"po{ch}") for ch in range(CH)]
                acc = lpool.tile([128, S], f32r, tag="lw", name="acc")
                for sk in range(NB):
                    # scores in two [128,1024] halves ping-ponging two PSUM
                    # slots so exp (ACT) overlaps the next scores matmuls.
                    ks = kT[h][:, sk * 128:(sk + 1) * 128]
                    ps0 = psS.tile([128, HF], f32, tag="ps", name="ps0")
                    for ch in (0, 1):
                        nc.tensor.matmul(
                            ps0[:, (ch % 2) * 512:(ch % 2) * 512 + 512],
                            lhsT=ks, rhs=qT[h][:, ch * 512:(ch + 1) * 512],
                            start=True, stop=True)
                    ps1 = psS.tile([128, HF], f32, tag="ps", name="ps1")
                    for ch in (2, 3):
                        nc.tensor.matmul(
                            ps1[:, (ch % 2) * 512:(ch % 2) * 512 + 512],
                            lhsT=ks, rhs=qT[h][:, ch * 512:(ch + 1) * 512],
                            start=True, stop=True)
                    e_t = epool.tile([128, S], f32r, tag="esb")
                    nc.scalar.activation(e_t[:, 0:HF], ps0[:],
                                         mybir.ActivationFunctionType.Exp,
                                         scale=float(SCALE))
                    nc.scalar.activation(e_t[:, HF:S], ps1[:],
                                         mybir.ActivationFunctionType.Exp,
                                         scale=float(SCALE))
                    vs = v_sb[sk][:, h * 128:(h + 1) * 128]
                    for ch in range(CH):
                        nc.tensor.matmul(
                            po[ch][:], lhsT=vs,
                            rhs=e_t[:, ch * 512:(ch + 1) * 512],
                            start=(sk == 0), stop=(sk == NB - 1))
                    if sk == 0:
                        nc.vector.tensor_copy(acc[:, 0:HF], e_t[:, 0:HF])
                        nc.vector.tensor_copy(acc[:, HF:S], e_t[:, HF:S])
                    else:
                        nc.vector.tensor_add(acc[:, 0:HF], acc[:, 0:HF], e_t[:, 0:HF])
                        nc.vector.tensor_add(acc[:, HF:S], acc[:, HF:S], e_t[:, HF:S])
                # l row-sums via ones-matmul (every psum row = the sum), then
                # fast reciprocal straight off PSUM.
                pl0 = psS.tile([128, HF], f32, tag="ps", name="pl0")
                nc.tensor.matmul(pl0[:, 0:512], lhsT=ones[:], rhs=acc[:, 0:512],
                                 start=True, stop=True)
                nc.tensor.matmul(pl0[:, 512:HF], lhsT=ones[:], rhs=acc[:, 512:HF],
                                 start=True, stop=True)
                pl1 = psS.tile([128, HF], f32, tag="ps", name="pl1")
                nc.tensor.matmul(pl1[:, 0:512], lhsT=ones[:], rhs=acc[:, HF:HF + 512],
                                 start=True, stop=True)
                nc.tensor.matmul(pl1[:, 512:HF], lhsT=ones[:], rhs=acc[:, HF + 512:S],
                                 start=True, stop=True)
                rb = lpool.tile([128, S], f32, tag="lw", name="rb")
                nc.vector.reciprocal_approx_fast(rb[:, 0:HF], pl0[:])
                nc.vector.reciprocal_approx_fast(rb[:, HF:S], pl1[:])
                rb16 = lpool.tile([128, S], f32r, tag="lw16", name="rb16")
                nc.scalar.copy(rb16[:], rb[:])
                aT = qk_pool.tile([128, S], f32r, tag=f"qT{h}", name=f"aT{h}")
                for ch in range(CH):
                    nc.scalar.copy(aT[:, ch * 512:(ch + 1) * 512], po[ch][:])
                nc.vector.tensor_mul(aT[:], aT[:], rb16[:])
                attnT.append(aT)

        # ---- phase D: o-projection partial ------------------------------
        with tc.tile_pool(name="osb", bufs=4) as opool, \
             tc.tile_pool(name="psD", bufs=2, space="PSUM") as psD:
            for nb in range(NB):
                pp = [psD.tile([128, 512], f32, tag=f"pp{ch}", name=f"pp{ch}") for ch in range(CH)]
                for cb in range(4):
                    for ch in range(CH):
                        nc.tensor.matmul(
                            pp[ch][:],
                            lhsT=wo_sb[cb][:, nb * 128:(nb + 1) * 128],
                            rhs=attnT[cb][:, ch * 512:(ch + 1) * 512],
                            start=(cb == 0), stop=(cb == 3))
                for ch in range(CH):
                    o_t = opool.tile([128, 512], f32, tag="osb")
                    nc.scalar.copy(o_t[:], pp[ch][:])
                    dma(outT[nb * 128:(nb + 1) * 128, ch * 512:(ch + 1) * 512], o_t[:])

    nc.compile()
    return nc


def _get_program():
    if "nc" not in _BUILT:
        _BUILT["nc"] = _build_program()
    return _BUILT["nc"]


def _reference_fallback(hidden_states, attention_mask, Wq, bq, Wk, bk, Wv, bv, Wo, bo):
    q = hidden_states @ Wq.T + bq
    k = hidden_states @ Wk.T + bk
    v = hidden_states @ Wv.T + bv
    q = q.reshape(B, S, H, HD).transpose(0, 2, 1, 3)
    k = k.reshape(B, S, H, HD).transpose(0, 2, 1, 3)
    v = v.reshape(B, S, H, HD).transpose(0, 2, 1, 3)
    scores = np.einsum("bhqd,bhkd->bhqk", q, k) / np.sqrt(np.float32(HD))
    scores = scores + attention_mask
    scores -= scores.max(axis=-1, keepdims=True)
    e = np.exp(scores)
    attn = e / e.sum(axis=-1, keepdims=True)
    out = np.einsum("bhqk,bhkd->bhqd", attn, v)
    out = out.transpose(0, 2, 1, 3).reshape(B, S, D)
    return (out @ Wo.T + bo).astype(np.float32)


def kernel(hidden_states, attention_mask, Wq, bq, Wk, bk, Wv, bv, Wo, bo):
    from concourse import bass_utils
    if MM_DTYPE == "bf16":
        import ml_dtypes
        in_dt = ml_dtypes.bfloat16
    elif MM_DTYPE == "fp16":
        in_dt = np.float16
    else:
        in_dt = np.float32

    hs = np.ascontiguousarray(np.asarray(hidden_states, dtype=np.float32))
    mask = np.asarray(attention_mask, dtype=np.float32)
    Wq = np.asarray(Wq, dtype=np.float32)
    Wk = np.asarray(Wk, dtype=np.float32)
    Wv = np.asarray(Wv, dtype=np.float32)
    Wo = np.asarray(Wo, dtype=np.float32)
    bq = np.asarray(bq, dtype=np.float32)
    bk = np.asarray(bk, dtype=np.float32)
    bv = np.asarray(bv, dtype=np.float32)
    bo = np.asarray(bo, dtype=np.float32)

    # Device program hardcodes zero mask / zero qkv biases (true for this
    # problem's setup_inputs); fall back to exact math if that ever changes.
    if mask.any() or bq.any() or bk.any() or bv.any():
        return _reference_fallback(hs, mask, Wq, bq, Wk, bk, Wv, bv, Wo, bo)

    nc = _get_program()

    hsT = [np.ascontiguousarray(hs[b].T).astype(in_dt) for b in range(B)]
    in_maps = []
    for c in range(8):
        b, g = divmod(c, 4)
        sl = slice(g * C, (g + 1) * C)
        in_maps.append({
            "hsT": hsT[b],
            "wq": np.ascontiguousarray(Wq[sl, :].T).astype(in_dt),
            "wk": np.ascontiguousarray(Wk[sl, :].T).astype(in_dt),
            "wv": np.ascontiguousarray(Wv[sl, :].T).astype(in_dt),
            "wo": np.ascontiguousarray(Wo[:, sl].T).astype(in_dt),
        })

    res = bass_utils.run_bass_kernel_spmd(nc, in_maps, core_ids=list(range(8)))

    out = np.empty((B, S, D), dtype=np.float32)
    for b in range(B):
        accT = res.results[b * 4 + 0]["outT"]
        for g in range(1, 4):
            accT = accT + res.results[b * 4 + g]["outT"]
        out[b] = accT.T + bo
    return out
